# Initial kernel scaffold
#
"""Trainium2 Bass kernel: 3-layer LSTM EEG classifier (B=64, C=64, T=1000, H=512, NC=5).

Sharding: data-parallel over batch -> 8 cores x 8 samples, weights replicated.

Per-core schedule (per LSTM layer):
  1. PROJ: xg = W_ih @ x_seq + b as a big throughput matmul (N=512 free dim),
     written to a DRAM scratch buffer in a gate-permuted "chunk" layout.
  2. TIME LOOP: 1000 sequential steps; per step 64 weight-stationary matmuls
     (16 gate chunks x 4 K-tiles, N=8) accumulate gates^T in 4 per-quarter
     PSUM banks, then per-quarter elementwise (sigmoid/tanh/cell update) on
     DVE+ACT overlapped with the next quarter's matmuls.
All matmul operands bf16 (fp32 accumulate); gates/cell state fp32.

Layouts (per core, P=128 partitions):
  gates^T tile [P, 128]: col = tau*32 + j*8 + b, where tau in {i,f,o,g} (gate
  type, host-permuted row order), j = hidden quarter (u = j*128 + p), b = batch.
  h^T / c^T tiles [P, 32]: col = j*8 + b.  h_seq keeps T+1 slots in SBUF (bf16).
"""

import numpy as np
import ml_dtypes

P = 128
B, C, T_FULL, H, L, NCLS = 64, 64, 1000, 512, 3, 5
G = 4 * H            # 2048 gate rows
KH = H // P          # 4 K-tiles over hidden
NCH = G // P         # 16 gate chunks
NCORES = 8
BL = B // NCORES     # 8 samples per core
U_DEF = 25           # time-loop unroll (body steps per hw loop iteration)

BF16 = ml_dtypes.bfloat16


def build_program(T=T_FULL, U=U_DEF):
    """Build the Bass program (single NeuronCore, run SPMD on 8 cores)."""
    import concourse.bass as bass
    import concourse.mybir as mybir
    import concourse.tile as tile
    from concourse.bass import ds

    assert T % U == 0
    f32 = mybir.dt.float32
    bf16 = mybir.dt.bfloat16
    AF = mybir.ActivationFunctionType

    nc = bass.Bass("TRN2", target_bir_lowering=False, debug=False)

    # ---------------- I/O ----------------
    xT = nc.dram_tensor("xT", [P, T * BL], bf16, kind="ExternalInput")
    wih0 = nc.dram_tensor("wih0", [P, G], bf16, kind="ExternalInput")
    wih12 = nc.dram_tensor("wih12", [2, KH, P, G], bf16, kind="ExternalInput")
    whh = nc.dram_tensor("whh", [L, KH, P, G], bf16, kind="ExternalInput")
    bias = nc.dram_tensor("bias", [L, P, NCH], f32, kind="ExternalInput")
    wfc = nc.dram_tensor("wfc", [KH, P, NCLS], bf16, kind="ExternalInput")
    bfc = nc.dram_tensor("bfc", [NCLS, 1], f32, kind="ExternalInput")
    out = nc.dram_tensor("out", [BL, NCLS], f32, kind="ExternalOutput")
    xg_d = [
        nc.dram_tensor(f"xg{i}", [P, T * P], f32, kind="Internal") for i in range(2)
    ]

    # ---------------- persistent SBUF ----------------
    whh_sb = nc.alloc_sbuf_tensor("whh_sb", [P, L * KH * G], bf16).ap()
    wih_sb = nc.alloc_sbuf_tensor("wih_sb", [P, 2 * KH * G], bf16).ap()
    wih0_sb = nc.alloc_sbuf_tensor("wih0_sb", [P, G], bf16).ap()
    x_sb = nc.alloc_sbuf_tensor("x_sb", [P, T * BL], bf16).ap()
    hseq = nc.alloc_sbuf_tensor("hseq", [P, (T + 1) * 4 * BL], bf16).ap()
    csb = nc.alloc_sbuf_tensor("csb", [P, 4 * BL], f32).ap()
    bias_sb = nc.alloc_sbuf_tensor("bias_sb", [P, L * NCH], f32).ap()
    wfc_sb = nc.alloc_sbuf_tensor("wfc_sb", [P, KH * NCLS], bf16).ap()
    bfc_sb = nc.alloc_sbuf_tensor("bfc_sb", [NCLS, 1], f32).ap()

    hv = hseq.rearrange("p (t x) -> p t x", x=4 * BL)  # [P, T+1, 32]

    chunks = [(t0, min(64, T - t0)) for t0 in range(0, T, 64)]

    with tile.TileContext(nc) as tc:
        with (
            tc.tile_pool(name="xgp", bufs=4) as xg_pool,
            tc.tile_pool(name="gp", bufs=3) as g_pool,
            tc.tile_pool(name="tmpp", bufs=4) as tmp_pool,
            tc.tile_pool(name="epp", bufs=3) as ep_pool,
            tc.tile_pool(name="psL", bufs=1, space="PSUM") as psL,
            tc.tile_pool(name="psP", bufs=2, space="PSUM") as psP,
        ):
            # ---- load weights/inputs into SBUF ----
            for l in range(L):
                for k in range(KH):
                    o = (l * KH + k) * G
                    nc.sync.dma_start(whh_sb[:, o : o + G], whh.ap()[l, k])
            for l in range(2):
                for k in range(KH):
                    o = (l * KH + k) * G
                    nc.sync.dma_start(wih_sb[:, o : o + G], wih12.ap()[l, k])
            nc.sync.dma_start(wih0_sb, wih0.ap())
            nc.sync.dma_start(x_sb, xT.ap())
            for l in range(L):
                nc.sync.dma_start(bias_sb[:, l * NCH : (l + 1) * NCH], bias.ap()[l])
            for k in range(KH):
                nc.sync.dma_start(wfc_sb[:, k * NCLS : (k + 1) * NCLS], wfc.ap()[k])
            nc.sync.dma_start(bfc_sb, bfc.ap())
            nc.vector.memset(hseq[:, 0 : 4 * BL], 0.0)  # h_{-1} = 0 slot

            for l in range(L):
                xg = xg_d[l % 2].ap()  # [P, T*P]
                xgv = xg.rearrange("p (t m) -> p t m", m=P)
                kt = 1 if l == 0 else KH

                # ---------- PROJ: xg = W_ih @ x + bias ----------
                for t0, tcnt in chunks:
                    ncols = tcnt * BL
                    for n in range(NCH):
                        ps = psP.tile([P, 512], f32, tag="proj")
                        for k in range(kt):
                            if l == 0:
                                lhsT = wih0_sb[:, n * P : (n + 1) * P]
                                rhs = x_sb[:, t0 * BL : (t0 + tcnt) * BL]
                            else:
                                o = ((l - 1) * KH + k) * G
                                lhsT = wih_sb[:, o + n * P : o + (n + 1) * P]
                                rhs = hv[:, t0 + 1 : t0 + 1 + tcnt, k * BL : (k + 1) * BL]
                            nc.tensor.matmul(
                                ps[:, :ncols], lhsT=lhsT, rhs=rhs,
                                start=(k == 0), stop=(k == kt - 1),
                            )
                        ep = ep_pool.tile([P, 512], f32, tag="ep")
                        nc.vector.tensor_scalar_add(
                            ep[:, :ncols], ps[:, :ncols],
                            bias_sb[:, l * NCH + n : l * NCH + n + 1],
                        )
                        nc.sync.dma_start(
                            xgv[:, t0 : t0 + tcnt, n * BL : (n + 1) * BL],
                            ep[:, :ncols].rearrange("p (t b) -> p t b", b=BL),
                        )

                # ---------- TIME LOOP ----------
                nc.vector.memset(csb, 0.0)
                nit = T // U
                with tc.For_i(
                    0, nit, 1, hint_engines=(mybir.EngineType.PE,)
                ) as it:
                    base_h = nc.snap(it * (U * 4 * BL))
                    base_x = nc.snap(it * (U * P))
                    for u in range(U):
                        xg_t = xg_pool.tile([P, P], f32, tag="xg")
                        nc.sync.dma_start(xg_t, xg[:, ds(base_x + u * P, P)])
                        g_sb = g_pool.tile([P, P], f32, tag="g")
                        gq4 = g_sb.rearrange("p (tau j b) -> p tau j b", tau=4, b=BL)
                        xq4 = xg_t.rearrange("p (tau j b) -> p tau j b", tau=4, b=BL)
                        rd = base_h + u * (4 * BL)        # h_{t-1} slot offset
                        wr = base_h + (u + 1) * (4 * BL)  # h_t slot offset
                        pss = []
                        for j in range(4):
                            ps = psL.tile([P, 4 * BL], f32, tag=f"q{j}")
                            pss.append(ps)
                            for tau in range(4):
                                nch = tau * 4 + j
                                wo = l * KH * G
                                for k in range(KH):
                                    nc.tensor.matmul(
                                        ps[:, tau * BL : (tau + 1) * BL],
                                        lhsT=whh_sb[
                                            :,
                                            wo + k * G + nch * P : wo + k * G + (nch + 1) * P,
                                        ],
                                        rhs=hseq[:, ds(rd + k * BL, BL)],
                                        start=(k == 0), stop=(k == KH - 1),
                                    )
                        for j in range(4):
                            psq = pss[j].rearrange("p (tau b) -> p tau b", b=BL)
                            gj = gq4[:, :, j, :]
                            nc.vector.tensor_add(gj, psq, xq4[:, :, j, :])
                            nc.scalar.activation(
                                gq4[:, 0:3, j, :], gq4[:, 0:3, j, :], AF.Sigmoid
                            )
                            nc.scalar.activation(
                                gq4[:, 3, j, :], gq4[:, 3, j, :], AF.Tanh
                            )
                            ig = tmp_pool.tile([P, BL], f32, tag="ig")
                            nc.vector.tensor_mul(ig, gq4[:, 0, j, :], gq4[:, 3, j, :])
                            cq = csb[:, j * BL : (j + 1) * BL]
                            nc.vector.tensor_mul(cq, gq4[:, 1, j, :], cq)
                            nc.vector.tensor_add(cq, cq, ig)
                            tc_ = tmp_pool.tile([P, BL], f32, tag="tc")
                            nc.scalar.activation(tc_, cq, AF.Tanh)
                            nc.vector.tensor_mul(
                                hseq[:, ds(wr + j * BL, BL)], gq4[:, 2, j, :], tc_
                            )

            # ---------- FC head ----------
            psf = psP.tile([NCLS, BL], f32, tag="fc")
            for k in range(KH):
                nc.tensor.matmul(
                    psf,
                    lhsT=wfc_sb[:, k * NCLS : (k + 1) * NCLS],
                    rhs=hv[:, T, k * BL : (k + 1) * BL],
                    start=(k == 0), stop=(k == KH - 1),
                )
            osb = tmp_pool.tile([NCLS, BL], f32, tag="osb")
            nc.vector.tensor_scalar_add(osb, psf, bfc_sb)
            nc.sync.dma_start(out.ap().rearrange("b c -> c b"), osb)

    return nc


# ---------------- host-side input prep ----------------

_GATE_PERM = np.concatenate(
    [np.arange(0, H), np.arange(H, 2 * H), np.arange(3 * H, 4 * H), np.arange(2 * H, 3 * H)]
)  # reorder gate blocks [i, f, g, o] -> [i, f, o, g]


def prep_weights(W_ih0, W_ih_rest, W_hh, b_ih, b_hh, W_fc, b_fc, T=T_FULL):
    """Host-side: permute/transpose/tile/cast weights into kernel input layout."""
    W_ih0 = np.asarray(W_ih0, np.float32)[_GATE_PERM]          # [G, C]
    wih0 = np.zeros((P, G), np.float32)
    wih0[:C] = W_ih0.T                                          # K-padded lhsT
    wih12 = np.stack(
        [np.asarray(W_ih_rest[i], np.float32)[_GATE_PERM].T.reshape(KH, P, G) for i in range(L - 1)]
    )                                                           # [2, KH, P, G]
    whh = np.stack(
        [np.asarray(W_hh[i], np.float32)[_GATE_PERM].T.reshape(KH, P, G) for i in range(L)]
    )                                                           # [L, KH, P, G]
    bsum = (np.asarray(b_ih, np.float32) + np.asarray(b_hh, np.float32))[:, _GATE_PERM]
    bias = np.ascontiguousarray(bsum.reshape(L, NCH, P).transpose(0, 2, 1))  # [L, P, NCH]
    wfc = np.asarray(W_fc, np.float32).T.reshape(KH, P, NCLS)   # [KH, P, NCLS]
    bfc = np.asarray(b_fc, np.float32).reshape(NCLS, 1)
    return {
        "wih0": wih0.astype(BF16),
        "wih12": wih12.astype(BF16),
        "whh": whh.astype(BF16),
        "bias": np.ascontiguousarray(bias, np.float32),
        "wfc": wfc.astype(BF16),
        "bfc": bfc,
    }


def prep_x_core(x_core, T=T_FULL):
    """x_core [BL, C, T] -> padded xT [P, T*BL] bf16 (col = t*BL + b)."""
    xt = np.zeros((P, T * BL), np.float32)
    xt[:C] = np.asarray(x_core, np.float32).transpose(1, 2, 0).reshape(C, T * BL)
    return xt.astype(BF16)


_CACHE = {}


def kernel(x, W_ih0, W_ih_rest, W_hh, b_ih, b_hh, W_fc, b_fc):
    from concourse.bass_utils import run_bass_kernel_spmd

    x = np.asarray(x, np.float32)
    wts = prep_weights(W_ih0, W_ih_rest, W_hh, b_ih, b_hh, W_fc, b_fc)
    in_maps = []
    for c in range(NCORES):
        m = dict(wts)
        m["xT"] = prep_x_core(x[c * BL : (c + 1) * BL])
        in_maps.append(m)

    if "nc" not in _CACHE:
        _CACHE["nc"] = build_program()
    res = run_bass_kernel_spmd(_CACHE["nc"], in_maps, core_ids=list(range(NCORES)))
    return np.concatenate([r["out"] for r in res.results], axis=0).astype(np.float32)


if __name__ == "__main__":
    rng = np.random.default_rng(0)
    s = 1.0 / np.sqrt(H)
    ins = dict(
        x=rng.standard_normal((B, C, T_FULL), dtype=np.float32),
        W_ih0=rng.uniform(-s, s, (G, C)).astype(np.float32),
        W_ih_rest=rng.uniform(-s, s, (L - 1, G, H)).astype(np.float32),
        W_hh=rng.uniform(-s, s, (L, G, H)).astype(np.float32),
        b_ih=rng.uniform(-s, s, (L, G)).astype(np.float32),
        b_hh=rng.uniform(-s, s, (L, G)).astype(np.float32),
        W_fc=rng.uniform(-s, s, (NCLS, H)).astype(np.float32),
        b_fc=rng.uniform(-s, s, (NCLS,)).astype(np.float32),
    )
    out = kernel(**ins)
    print(out.shape, out.dtype, np.abs(out).max())


# revision 8
# speedup vs baseline: 27.1636x; 27.1636x over previous
"""Trainium2 Bass kernel: 3-layer LSTM EEG classifier (B=64, C=64, T=1000, H=512, NC=5).

Sharding: data-parallel over batch -> 8 cores x 8 samples, weights replicated.

Per-core schedule (per LSTM layer):
  1. PROJ: xg = W_ih @ x_seq + b as a big throughput matmul (N=512 free dim),
     written to a DRAM scratch buffer in a gate-permuted "chunk" layout.
  2. TIME LOOP: 1000 sequential steps; per step 64 weight-stationary matmuls
     (16 gate chunks x 4 K-tiles, N=8) accumulate gates^T in 4 per-quarter
     PSUM banks, then per-quarter elementwise (sigmoid/tanh/cell update) on
     DVE+ACT overlapped with the next quarter's matmuls.
All matmul operands bf16 (fp32 accumulate); gates/cell state fp32.

Layouts (per core, P=128 partitions):
  gates^T tile [P, 128]: col = tau*32 + j*8 + b, where tau in {i,f,o,g} (gate
  type, host-permuted row order), j = hidden quarter (u = j*128 + p), b = batch.
  h^T / c^T tiles [P, 32]: col = j*8 + b.  h_seq keeps T+1 slots in SBUF (bf16).
"""

import numpy as np
import ml_dtypes

P = 128
B, C, T_FULL, H, L, NCLS = 64, 64, 1000, 512, 3, 5
G = 4 * H            # 2048 gate rows
KH = H // P          # 4 K-tiles over hidden
NCH = G // P         # 16 gate chunks
NCORES = 8
BL = B // NCORES     # 8 samples per core
U_DEF = 20           # time-loop unroll (body steps per hw loop iteration)

BF16 = ml_dtypes.bfloat16


def build_program(T=T_FULL, U=U_DEF):
    """Build the Bass program (single NeuronCore, run SPMD on 8 cores)."""
    import concourse.bass as bass
    import concourse.mybir as mybir
    import concourse.tile as tile
    from concourse import bacc
    from concourse.bass import ds

    assert T % U == 0
    f32 = mybir.dt.float32
    bf16 = mybir.dt.bfloat16
    AF = mybir.ActivationFunctionType

    nc = bacc.Bacc("TRN2", target_bir_lowering=False, debug=False)

    # ---------------- I/O ----------------
    xT = nc.dram_tensor("xT", [P, T * BL], bf16, kind="ExternalInput")
    wih0 = nc.dram_tensor("wih0", [P, G], bf16, kind="ExternalInput")
    wih12 = nc.dram_tensor("wih12", [2, KH, P, G], bf16, kind="ExternalInput")
    whh = nc.dram_tensor("whh", [L, KH, P, G], bf16, kind="ExternalInput")
    bias = nc.dram_tensor("bias", [L, P, NCH], f32, kind="ExternalInput")
    wfc = nc.dram_tensor("wfc", [KH, P, NCLS], bf16, kind="ExternalInput")
    bfc = nc.dram_tensor("bfc", [NCLS, 1], f32, kind="ExternalInput")
    out = nc.dram_tensor("out", [BL, NCLS], f32, kind="ExternalOutput")
    xg_d = [
        nc.dram_tensor(f"xg{i}", [P, T * P], f32, kind="Internal") for i in range(2)
    ]

    # ---------------- persistent SBUF ----------------
    whh_sb = nc.alloc_sbuf_tensor("whh_sb", [P, L * KH * G], bf16).ap()
    wih_sb = nc.alloc_sbuf_tensor("wih_sb", [P, 2 * KH * G], bf16).ap()
    wih0_sb = nc.alloc_sbuf_tensor("wih0_sb", [P, G], bf16).ap()
    x_sb = nc.alloc_sbuf_tensor("x_sb", [P, T * BL], bf16).ap()
    hseq = nc.alloc_sbuf_tensor("hseq", [P, (T + 1) * 4 * BL], bf16).ap()
    csb = nc.alloc_sbuf_tensor("csb", [P, 4 * BL], f32).ap()
    bias_sb = nc.alloc_sbuf_tensor("bias_sb", [P, L * NCH], f32).ap()
    wfc_sb = nc.alloc_sbuf_tensor("wfc_sb", [P, KH * NCLS], bf16).ap()
    bfc_sb = nc.alloc_sbuf_tensor("bfc_sb", [NCLS, 1], f32).ap()
    # static staging for the time loop: all per-step APs stay register-free
    # (dynamic-offset APs burn a per-engine register per instruction per loop
    # body, budget ~12 — only the few staging DMAs below use dynamic offsets)
    stage_h = nc.alloc_sbuf_tensor("stage_h", [P, U * 4 * BL], bf16).ap()
    stage_xg = nc.alloc_sbuf_tensor("stage_xg", [P, U * P], f32).ap()

    hv = hseq.rearrange("p (t x) -> p t x", x=4 * BL)  # [P, T+1, 32]

    chunks = [(t0, min(64, T - t0)) for t0 in range(0, T, 64)]

    with tile.TileContext(nc) as tc:
        with (
            tc.tile_pool(name="gp", bufs=3) as g_pool,
            tc.tile_pool(name="tmpp", bufs=4) as tmp_pool,
            tc.tile_pool(name="epp", bufs=3) as ep_pool,
            tc.tile_pool(name="psL", bufs=1, space="PSUM") as psL,
            tc.tile_pool(name="psP", bufs=2, space="PSUM") as psP,
        ):
            # ---- load weights/inputs into SBUF ----
            for l in range(L):
                for k in range(KH):
                    o = (l * KH + k) * G
                    nc.sync.dma_start(whh_sb[:, o : o + G], whh.ap()[l, k])
            for l in range(2):
                for k in range(KH):
                    o = (l * KH + k) * G
                    nc.sync.dma_start(wih_sb[:, o : o + G], wih12.ap()[l, k])
            nc.sync.dma_start(wih0_sb, wih0.ap())
            nc.sync.dma_start(x_sb, xT.ap())
            for l in range(L):
                nc.sync.dma_start(bias_sb[:, l * NCH : (l + 1) * NCH], bias.ap()[l])
            for k in range(KH):
                nc.sync.dma_start(wfc_sb[:, k * NCLS : (k + 1) * NCLS], wfc.ap()[k])
            nc.sync.dma_start(bfc_sb, bfc.ap())
            nc.vector.memset(hseq[:, 0 : 4 * BL], 0.0)  # h_{-1} = 0 slot

            for l in range(L):
                xg = xg_d[l % 2].ap()  # [P, T*P]
                xgv = xg.rearrange("p (t m) -> p t m", m=P)
                kt = 1 if l == 0 else KH

                # ---------- PROJ: xg = W_ih @ x + bias ----------
                for t0, tcnt in chunks:
                    ncols = tcnt * BL
                    for n in range(NCH):
                        ps = psP.tile([P, 512], f32, tag="proj")
                        for k in range(kt):
                            if l == 0:
                                lhsT = wih0_sb[:, n * P : (n + 1) * P]
                                rhs = x_sb[:, t0 * BL : (t0 + tcnt) * BL]
                            else:
                                o = ((l - 1) * KH + k) * G
                                lhsT = wih_sb[:, o + n * P : o + (n + 1) * P]
                                rhs = hv[:, t0 + 1 : t0 + 1 + tcnt, k * BL : (k + 1) * BL]
                            nc.tensor.matmul(
                                ps[:, :ncols], lhsT=lhsT, rhs=rhs,
                                start=(k == 0), stop=(k == kt - 1),
                            )
                        ep = ep_pool.tile([P, 512], f32, tag="ep")
                        nc.vector.tensor_scalar_add(
                            ep[:, :ncols], ps[:, :ncols],
                            bias_sb[:, l * NCH + n : l * NCH + n + 1],
                        )
                        nc.sync.dma_start(
                            xgv[:, t0 : t0 + tcnt, n * BL : (n + 1) * BL],
                            ep[:, :ncols].rearrange("p (t b) -> p t b", b=BL),
                        )

                # ---------- TIME LOOP ----------
                nc.vector.memset(csb, 0.0)
                # h_{-1} = 0: slot U-1 of stage_h is what step u=0 reads
                nc.vector.memset(stage_h[:, (U - 1) * 4 * BL : U * 4 * BL], 0.0)
                nit = T // U
                XCH = 4  # xg staging DMA chunks per body
                assert U % XCH == 0
                with tc.For_i(
                    0, nit, 1, hint_engines=(mybir.EngineType.PE,)
                ) as it:
                    for c in range(XCH):
                        w = (U // XCH) * P
                        nc.sync.dma_start(
                            stage_xg[:, c * w : (c + 1) * w],
                            xg[:, ds(it * (U * P) + c * w, w)],
                        )
                    for u in range(U):
                        xg_t = stage_xg[:, u * P : (u + 1) * P]
                        g_sb = g_pool.tile([P, P], f32, tag="g")
                        gq4 = g_sb.rearrange("p (tau j b) -> p tau j b", tau=4, b=BL)
                        xq4 = xg_t.rearrange("p (tau j b) -> p tau j b", tau=4, b=BL)
                        rd = ((u - 1) % U) * (4 * BL)  # h_{t-1} staging slot
                        wr = u * (4 * BL)              # h_t staging slot
                        pss = []
                        for j in range(4):
                            ps = psL.tile([P, 4 * BL], f32, tag=f"q{j}", name=f"psq{j}")
                            pss.append(ps)
                            for tau in range(4):
                                nch = tau * 4 + j
                                wo = l * KH * G
                                for k in range(KH):
                                    nc.tensor.matmul(
                                        ps[:, tau * BL : (tau + 1) * BL],
                                        lhsT=whh_sb[
                                            :,
                                            wo + k * G + nch * P : wo + k * G + (nch + 1) * P,
                                        ],
                                        rhs=stage_h[:, rd + k * BL : rd + (k + 1) * BL],
                                        start=(k == 0), stop=(k == KH - 1),
                                    )
                        for j in range(4):
                            psq = pss[j].rearrange("p (tau b) -> p tau b", b=BL)
                            gj = gq4[:, :, j, :]
                            nc.vector.tensor_add(gj, psq, xq4[:, :, j, :])
                            nc.scalar.activation(
                                gq4[:, 0:3, j, :], gq4[:, 0:3, j, :], AF.Sigmoid
                            )
                            nc.scalar.activation(
                                gq4[:, 3, j, :], gq4[:, 3, j, :], AF.Tanh
                            )
                            ig = tmp_pool.tile([P, BL], f32, tag="ig")
                            nc.vector.tensor_mul(ig, gq4[:, 0, j, :], gq4[:, 3, j, :])
                            cq = csb[:, j * BL : (j + 1) * BL]
                            nc.vector.tensor_mul(cq, gq4[:, 1, j, :], cq)
                            nc.vector.tensor_add(cq, cq, ig)
                            tc_ = tmp_pool.tile([P, BL], f32, tag="tc")
                            nc.scalar.activation(tc_, cq, AF.Tanh)
                            nc.vector.tensor_mul(
                                stage_h[:, wr + j * BL : wr + (j + 1) * BL],
                                gq4[:, 2, j, :], tc_,
                            )
                    # persist this body's h_t slots into hseq[it*U+1 .. it*U+U]
                    nc.sync.dma_start(
                        hseq[:, ds(it * (U * 4 * BL) + 4 * BL, U * 4 * BL)], stage_h
                    )

            # ---------- FC head ----------
            psf = psP.tile([NCLS, BL], f32, tag="fc")
            for k in range(KH):
                nc.tensor.matmul(
                    psf,
                    lhsT=wfc_sb[:, k * NCLS : (k + 1) * NCLS],
                    rhs=hv[:, T, k * BL : (k + 1) * BL],
                    start=(k == 0), stop=(k == KH - 1),
                )
            osb = tmp_pool.tile([NCLS, BL], f32, tag="osb")
            nc.vector.tensor_scalar_add(osb, psf, bfc_sb)
            nc.sync.dma_start(out.ap().rearrange("b c -> c b"), osb)

    nc.compile()
    return nc


# ---------------- host-side input prep ----------------

_GATE_PERM = np.concatenate(
    [np.arange(0, H), np.arange(H, 2 * H), np.arange(3 * H, 4 * H), np.arange(2 * H, 3 * H)]
)  # reorder gate blocks [i, f, g, o] -> [i, f, o, g]


def prep_weights(W_ih0, W_ih_rest, W_hh, b_ih, b_hh, W_fc, b_fc, T=T_FULL):
    """Host-side: permute/transpose/tile/cast weights into kernel input layout."""
    W_ih0 = np.asarray(W_ih0, np.float32)[_GATE_PERM]          # [G, C]
    wih0 = np.zeros((P, G), np.float32)
    wih0[:C] = W_ih0.T                                          # K-padded lhsT
    wih12 = np.stack(
        [np.asarray(W_ih_rest[i], np.float32)[_GATE_PERM].T.reshape(KH, P, G) for i in range(L - 1)]
    )                                                           # [2, KH, P, G]
    whh = np.stack(
        [np.asarray(W_hh[i], np.float32)[_GATE_PERM].T.reshape(KH, P, G) for i in range(L)]
    )                                                           # [L, KH, P, G]
    bsum = (np.asarray(b_ih, np.float32) + np.asarray(b_hh, np.float32))[:, _GATE_PERM]
    bias = np.ascontiguousarray(bsum.reshape(L, NCH, P).transpose(0, 2, 1))  # [L, P, NCH]
    wfc = np.asarray(W_fc, np.float32).T.reshape(KH, P, NCLS)   # [KH, P, NCLS]
    bfc = np.asarray(b_fc, np.float32).reshape(NCLS, 1)
    return {
        "wih0": wih0.astype(BF16),
        "wih12": wih12.astype(BF16),
        "whh": whh.astype(BF16),
        "bias": np.ascontiguousarray(bias, np.float32),
        "wfc": wfc.astype(BF16),
        "bfc": bfc,
    }


def prep_x_core(x_core, T=T_FULL):
    """x_core [BL, C, T] -> padded xT [P, T*BL] bf16 (col = t*BL + b)."""
    xt = np.zeros((P, T * BL), np.float32)
    xt[:C] = np.asarray(x_core, np.float32).transpose(1, 2, 0).reshape(C, T * BL)
    return xt.astype(BF16)


_CACHE = {}


def kernel(x, W_ih0, W_ih_rest, W_hh, b_ih, b_hh, W_fc, b_fc):
    from concourse.bass_utils import run_bass_kernel_spmd

    x = np.asarray(x, np.float32)
    wts = prep_weights(W_ih0, W_ih_rest, W_hh, b_ih, b_hh, W_fc, b_fc)
    in_maps = []
    for c in range(NCORES):
        m = dict(wts)
        m["xT"] = prep_x_core(x[c * BL : (c + 1) * BL])
        in_maps.append(m)

    if "nc" not in _CACHE:
        _CACHE["nc"] = build_program()
    res = run_bass_kernel_spmd(_CACHE["nc"], in_maps, core_ids=list(range(NCORES)))
    return np.concatenate([r["out"] for r in res.results], axis=0).astype(np.float32)


if __name__ == "__main__":
    rng = np.random.default_rng(0)
    s = 1.0 / np.sqrt(H)
    ins = dict(
        x=rng.standard_normal((B, C, T_FULL), dtype=np.float32),
        W_ih0=rng.uniform(-s, s, (G, C)).astype(np.float32),
        W_ih_rest=rng.uniform(-s, s, (L - 1, G, H)).astype(np.float32),
        W_hh=rng.uniform(-s, s, (L, G, H)).astype(np.float32),
        b_ih=rng.uniform(-s, s, (L, G)).astype(np.float32),
        b_hh=rng.uniform(-s, s, (L, G)).astype(np.float32),
        W_fc=rng.uniform(-s, s, (NCLS, H)).astype(np.float32),
        b_fc=rng.uniform(-s, s, (NCLS,)).astype(np.float32),
    )
    out = kernel(**ins)
    print(out.shape, out.dtype, np.abs(out).max())
